# revision 2
# baseline (speedup 1.0000x reference)
"""BarlowTwinsLoss on 8 Trainium2 NeuronCores.

Math: with xs = standardize(X1), ys = standardize(X2) (per-feature batch
stats, ddof=1), C = cos-sim matrix of rows: C[i,j] = u_i . v_j where
u_i = xs_i/|xs_i|, v_j = ys_j/|ys_j|.  The loss only needs
  inv_term = (N - sum_i C_ii)/N
  red_term = LAM/N * (sum_ij C_ij^2 - sum_i C_ii^2)
and sum_ij C_ij^2 = <U^T U, V^T V>_F, which collapses the O(N^2 F) problem
to O(N F^2): two [F,F] Gram matrices.

Distribution: rows sharded 8 ways.  Per core: partial column moments ->
AllReduce (tiny) -> standardize local rows -> local Gram partials A_c, B_c
[64,64] + diag partials -> ReduceScatter of (A|B|sp|spp) so core k holds
8 feature-rows of the global A and B -> per-core partial scalar loss.
Host sums the 8 partial-loss scalars (the "all-reduce the scalar partial
losses" step of the sharding hint).
"""

import numpy as np

N_CORES = 8
N_TOTAL = 16384
F = 64
ROWS = N_TOTAL // N_CORES  # 2048 rows per core
J = 16                     # free-dim row-chunks per partition: 128 * 16 = 2048
LAM = 0.2

_BUILT = {}


def _build_bass():
    import concourse.bacc as bacc
    import concourse.mybir as mybir
    import concourse.tile as tile

    fp32 = mybir.dt.float32
    bf16 = mybir.dt.bfloat16
    mult = mybir.AluOpType.mult
    add = mybir.AluOpType.add
    subtract = mybir.AluOpType.subtract
    AX = mybir.AxisListType.X

    nc = bacc.Bacc(
        "TRN2", target_bir_lowering=False, debug=False, num_devices=N_CORES
    )

    x1_d = nc.dram_tensor("x1", [ROWS, F], fp32, kind="ExternalInput")
    x2_d = nc.dram_tensor("x2", [ROWS, F], fp32, kind="ExternalInput")
    out_d = nc.dram_tensor("out", [1, 1], fp32, kind="ExternalOutput")

    rg = [list(range(N_CORES))]
    Nf = float(N_TOTAL)

    with tile.TileContext(nc) as tc:
        with (
            tc.tile_pool(name="sb", bufs=1) as sb,
            tc.tile_pool(name="ps", bufs=1, space="PSUM") as ps,
            tc.tile_pool(name="dram", bufs=1, space="DRAM") as dram,
        ):
            # ---- constants ----
            ones_bf = sb.tile([128, 1], bf16)
            ones_fr = sb.tile([1, 128], fp32)   # row of ones (K=1 bcast matmuls)
            ones_fc = sb.tile([128, 1], fp32)   # column of ones (partition folds)
            nc.vector.memset(ones_bf[:], 1.0)
            nc.vector.memset(ones_fr[:], 1.0)
            nc.vector.memset(ones_fc[:], 1.0)

            # ---- load inputs: [2048,64] -> [128 partitions, 16 chunks, 64] ----
            # partition p holds rows p*16 .. p*16+15 (4KB contiguous per partition)
            x1f = sb.tile([128, J, F], fp32)
            x2f = sb.tile([128, J, F], fp32)
            nc.sync.dma_start(x1f[:], x1_d.ap().rearrange("(p j) f -> p j f", p=128))
            nc.sync.dma_start(x2f[:], x2_d.ap().rearrange("(p j) f -> p j f", p=128))

            # ---- bf16 casts + squares (squares from f32 source, on ACT) ----
            x1b = sb.tile([128, J, F], bf16)
            x2b = sb.tile([128, J, F], bf16)
            sq1 = sb.tile([128, J, F], bf16)
            sq2 = sb.tile([128, J, F], bf16)
            nc.vector.tensor_copy(x1b[:], x1f[:])
            nc.vector.tensor_copy(x2b[:], x2f[:])
            nc.scalar.square(sq1[:], x1f[:])
            nc.scalar.square(sq2[:], x2f[:])

            # ---- column-stat partials: fold j 16->1, then ones-matmul over partitions
            # statcat[:, q*64:(q+1)*64] = j-folded quantity q in (x1, sq1, x2, sq2)
            statcat = sb.tile([128, 4 * F], bf16)
            for q, src in enumerate((x1b, sq1, x2b, sq2)):
                fa = sb.tile([128, 8, F], bf16, tag="folda", bufs=2)
                fb = sb.tile([128, 4, F], bf16, tag="foldb", bufs=2)
                fc = sb.tile([128, 2, F], bf16, tag="foldc", bufs=2)
                nc.vector.tensor_add(fa[:], src[:, 0:8, :], src[:, 8:16, :])
                nc.vector.tensor_add(fb[:], fa[:, 0:4, :], fa[:, 4:8, :])
                nc.vector.tensor_add(fc[:], fb[:, 0:2, :], fb[:, 2:4, :])
                nc.vector.tensor_add(
                    statcat[:, q * F:(q + 1) * F], fc[:, 0, :], fc[:, 1, :]
                )
            stat_ps = ps.tile([1, 4 * F], fp32)
            nc.tensor.matmul(stat_ps[:], ones_bf[:], statcat[:], start=True, stop=True)

            # ---- AllReduce the 256 floats of raw column moments ----
            stat_sb = sb.tile([1, 4 * F], fp32)
            nc.vector.tensor_copy(stat_sb[:], stat_ps[:])
            ar_in = dram.tile([1, 4 * F], fp32)
            ar_out = dram.tile([1, 4 * F], fp32, addr_space="Shared")
            nc.sync.dma_start(ar_in[:], stat_sb[:])
            nc.gpsimd.collective_compute(
                "AllReduce",
                add,
                replica_groups=rg,
                ins=[ar_in.opt()],
                outs=[ar_out.opt()],
            )
            stats = sb.tile([1, 4 * F], fp32)
            nc.sync.dma_start(stats[:], ar_out[:])

            # ---- mu/inv_sd per input; pack [isd1|mu*isd1|isd2|mu*isd2] ----
            bsrc = sb.tile([1, 4 * F], fp32)
            for i in range(2):
                s1 = stats[:, (2 * i) * F:(2 * i + 1) * F]
                s2 = stats[:, (2 * i + 1) * F:(2 * i + 2) * F]
                mu = sb.tile([1, F], fp32, tag="mu", bufs=2)
                mnn = sb.tile([1, F], fp32, tag="mnn", bufs=2)
                var = sb.tile([1, F], fp32, tag="var", bufs=2)
                sd = sb.tile([1, F], fp32, tag="sd", bufs=2)
                nc.vector.tensor_scalar(mu[:], s1, 1.0 / Nf, None, mult)
                # mnn = mu^2 * N/(N-1)
                nc.vector.scalar_tensor_tensor(
                    mnn[:], mu[:], Nf / (Nf - 1.0), mu[:], mult, mult
                )
                # var = s2/(N-1) - mnn
                nc.vector.scalar_tensor_tensor(
                    var[:], s2, 1.0 / (Nf - 1.0), mnn[:], mult, subtract
                )
                nc.scalar.sqrt(sd[:], var[:])
                isd = bsrc[:, (2 * i) * F:(2 * i + 1) * F]
                nc.vector.reciprocal(isd, sd[:])
                nc.vector.tensor_mul(
                    bsrc[:, (2 * i + 1) * F:(2 * i + 2) * F], mu[:], isd
                )

            # ---- broadcast stats across partitions via K=1 ones-matmul ----
            bc_ps = ps.tile([128, 4 * F], fp32)
            nc.tensor.matmul(
                bc_ps[:], ones_fr[:, 0:128], bsrc[:], start=True, stop=True
            )
            bcb = sb.tile([128, 4 * F], bf16)
            nc.vector.tensor_copy(bcb[:], bc_ps[:])
            ISD1 = bcb[:, 0 * F:1 * F].unsqueeze(1).broadcast_to([128, J, F])
            MIS1 = bcb[:, 1 * F:2 * F].unsqueeze(1).broadcast_to([128, J, F])
            ISD2 = bcb[:, 2 * F:3 * F].unsqueeze(1).broadcast_to([128, J, F])
            MIS2 = bcb[:, 3 * F:4 * F].unsqueeze(1).broadcast_to([128, J, F])

            # ---- standardize: xs = x*isd - mu*isd ----
            xs1 = sb.tile([128, J, F], bf16)
            xs2 = sb.tile([128, J, F], bf16)
            z1 = sb.tile([128, J, F], bf16, tag="zt", bufs=2)
            z2 = sb.tile([128, J, F], bf16, tag="zt", bufs=2)
            nc.vector.tensor_mul(z1[:], x1b[:], ISD1)
            nc.vector.tensor_sub(xs1[:], z1[:], MIS1)
            nc.vector.tensor_mul(z2[:], x2b[:], ISD2)
            nc.vector.tensor_sub(xs2[:], z2[:], MIS2)

            # ---- row norms^2, cross dot, 1/r^2 scaling ----
            sqs = sb.tile([128, J, F], bf16, tag="sqs", bufs=2)
            r2_1 = sb.tile([128, J], fp32)
            r2_2 = sb.tile([128, J], fp32)
            mnum = sb.tile([128, J], fp32)
            nc.vector.tensor_mul(sqs[:], xs1[:], xs1[:])
            nc.vector.tensor_reduce(r2_1[:], sqs[:], AX, add)
            sqs2 = sb.tile([128, J, F], bf16, tag="sqs", bufs=2)
            nc.vector.tensor_mul(sqs2[:], xs2[:], xs2[:])
            nc.vector.tensor_reduce(r2_2[:], sqs2[:], AX, add)
            mts = sb.tile([128, J, F], bf16, tag="sqs", bufs=2)
            nc.vector.tensor_mul(mts[:], xs1[:], xs2[:])
            nc.vector.tensor_reduce(mnum[:], mts[:], AX, add)

            w1 = sb.tile([128, J], fp32)
            w2 = sb.tile([128, J], fp32)
            nc.vector.reciprocal(w1[:], r2_1[:])
            nc.vector.reciprocal(w2[:], r2_2[:])
            w1b = sb.tile([128, J], bf16)
            w2b = sb.tile([128, J], bf16)
            nc.vector.tensor_copy(w1b[:], w1[:])
            nc.vector.tensor_copy(w2b[:], w2[:])
            xw1 = sb.tile([128, J, F], bf16)
            xw2 = sb.tile([128, J, F], bf16)
            nc.vector.tensor_mul(
                xw1[:], xs1[:], w1b[:].unsqueeze(2).broadcast_to([128, J, F])
            )
            nc.vector.tensor_mul(
                xw2[:], xs2[:], w2b[:].unsqueeze(2).broadcast_to([128, J, F])
            )

            # ---- diagonal terms: p_i = mnum_i / sqrt(r2_1 r2_2) ----
            qq = sb.tile([128, J], fp32)
            sqq = sb.tile([128, J], fp32)
            rq = sb.tile([128, J], fp32)
            pp = sb.tile([128, J], fp32)
            pcols = sb.tile([128, 2], fp32)
            psc = sb.tile([128, J], fp32)
            nc.vector.tensor_mul(qq[:], r2_1[:], r2_2[:])
            nc.scalar.sqrt(sqq[:], qq[:])
            nc.vector.reciprocal(rq[:], sqq[:])
            nc.vector.tensor_mul(pp[:], mnum[:], rq[:])
            nc.vector.tensor_reduce(pcols[:, 0:1], pp[:], AX, add)
            # psc = p^2, accum -> pcols[:,1]
            nc.vector.scalar_tensor_tensor(
                psc[:], pp[:], 1.0, pp[:], mult, mult, accum_out=pcols[:, 1:2]
            )
            sp_ps = ps.tile([1, 2], fp32)
            nc.tensor.matmul(sp_ps[:], ones_fc[:], pcols[:], start=True, stop=True)
            sp_sb = sb.tile([1, 2], fp32)
            nc.vector.tensor_copy(sp_sb[:], sp_ps[:])
            # broadcast (sp, spp) to 64 rows for the ReduceScatter payload
            spb_ps = ps.tile([F, 2], fp32)
            nc.tensor.matmul(
                spb_ps[:], ones_fr[:, 0:F], sp_sb[:], start=True, stop=True
            )

            # ---- Gram partials A = xs1^T (xs1 * w1), B likewise ----
            gramA = ps.tile([F, F], fp32)
            gramB = ps.tile([F, F], fp32)
            for j in range(J):
                nc.tensor.matmul(
                    gramA[:], xs1[:, j, :], xw1[:, j, :],
                    start=(j == 0), stop=(j == J - 1),
                )
            for j in range(J):
                nc.tensor.matmul(
                    gramB[:], xs2[:, j, :], xw2[:, j, :],
                    start=(j == 0), stop=(j == J - 1),
                )

            # ---- ReduceScatter payload: [64 rows, A_f | B_f | sp | spp] ----
            ab_sb = sb.tile([F, 2 * F + 2], fp32)
            nc.vector.tensor_copy(ab_sb[:, 0:F], gramA[:])
            nc.vector.tensor_copy(ab_sb[:, F:2 * F], gramB[:])
            nc.vector.tensor_copy(ab_sb[:, 2 * F:2 * F + 2], spb_ps[:])
            rs_in = dram.tile([F, 2 * F + 2], fp32)
            rs_out = dram.tile([F // N_CORES, 2 * F + 2], fp32)
            nc.sync.dma_start(rs_in[:], ab_sb[:])
            nc.gpsimd.collective_compute(
                "ReduceScatter",
                add,
                replica_groups=rg,
                ins=[rs_in.opt()],
                outs=[rs_out.opt()],
            )
            S = F // N_CORES  # 8 feature-rows of the global Grams per core
            rs_sb = sb.tile([S, 2 * F + 2], fp32)
            nc.sync.dma_start(rs_sb[:], rs_out[:])

            # ---- per-core partial loss ----
            abm = sb.tile([S, F], fp32)
            abf = sb.tile([S, 1], fp32)
            nc.vector.tensor_mul(abm[:], rs_sb[:, 0:F], rs_sb[:, F:2 * F])
            nc.vector.tensor_reduce(abf[:], abm[:], AX, add)
            dot_ps = ps.tile([1, 1], fp32)
            nc.tensor.matmul(
                dot_ps[:], ones_fc[0:S, :], abf[:], start=True, stop=True
            )
            # t1 = (1 - sp/N)/8 ; t2 = (LAM/N)*dot - (LAM/N)*spp/8
            t1 = sb.tile([1, 1], fp32)
            t2 = sb.tile([1, 1], fp32)
            spp_s = sb.tile([1, 1], fp32)
            loss = sb.tile([1, 1], fp32)
            nc.vector.tensor_scalar(
                t1[:], rs_sb[0:1, 2 * F:2 * F + 1],
                -1.0 / (Nf * N_CORES), 1.0 / N_CORES, mult, add,
            )
            nc.vector.tensor_scalar(
                spp_s[:], rs_sb[0:1, 2 * F + 1:2 * F + 2],
                LAM / (Nf * N_CORES), None, mult,
            )
            nc.vector.scalar_tensor_tensor(
                t2[:], dot_ps[:], LAM / Nf, spp_s[:], mult, subtract
            )
            nc.vector.tensor_add(loss[:], t1[:], t2[:])
            nc.sync.dma_start(out_d.ap(), loss[:])

    nc.compile()
    return nc


def _get_nc():
    if "nc" not in _BUILT:
        _BUILT["nc"] = _build_bass()
    return _BUILT["nc"]


def kernel(X1, X2):
    from concourse import bass_utils

    X1 = np.ascontiguousarray(np.asarray(X1), dtype=np.float32)
    X2 = np.ascontiguousarray(np.asarray(X2), dtype=np.float32)
    assert X1.shape == (N_TOTAL, F) and X2.shape == (N_TOTAL, F)

    nc = _get_nc()
    in_maps = [
        {
            "x1": X1[k * ROWS:(k + 1) * ROWS],
            "x2": X2[k * ROWS:(k + 1) * ROWS],
        }
        for k in range(N_CORES)
    ]
    res = bass_utils.run_bass_kernel_spmd(nc, in_maps, list(range(N_CORES)))
    partials = [np.float32(r["out"][0, 0]) for r in res.results]
    return np.float32(np.sum(np.asarray(partials, dtype=np.float64)))
